# Initial kernel scaffold
#
"""Depthwise cross-correlation (per-sample dynamic kernel) on 8 Trainium2 cores.

reference: out[b,i,j,c] = sum_{di,dj} search[b,i+di,j+dj,c] * template[b,di,dj,c]
  search [64,31,31,256] f32, template [64,7,7,256] f32 -> out [64,25,25,256] f32

Strategy (pure data parallel, 8 samples/core, no collectives):
- Host marshals channel-major blobs, one per (sample): for each channel-half
  [128 part, 2 x (search 961 | raw template 49 | diag tiles for PE taps)].
- PE path: out_chunk = sum_k diag(t_k) @ shift_k(S) accumulated in PSUM
  (the only way a systolic array does depthwise). float32r dtype: fp32 bits
  at bf16-like speed, ~1e-4 rel err. fp32r needs even innermost counts and
  dst partition 0 -> full-height 128 diags, output window padded 25->26.
- ACT+DVE path: the last N_DVE taps run as ScalarE per-channel multiplies
  (activation Copy with per-partition scale AP) + VectorE accumulate adds,
  in parallel with the PE stream; folded with the PSUM result at evacuation.
- Output rows split 13+12 so each PSUM accumulation target is one bank with
  N>=256 (below 256 float32r drops to quarter rate).
- A post-pass splits multi-wait instructions (walrus allows one sync-wait
  per instruction) into single-wait NoOp carriers.
"""
import sys

sys.path.insert(0, "/opt/trn_rl_repo")

import numpy as np
import concourse.bass as bass
import concourse.mybir as mybir
import concourse.tile as tile
from concourse.bass_utils import run_bass_kernel_spmd

B = 64
X, K, OUT = 31, 7, 25
CH = 256
C = 128                      # channels per half (partition dim)
N_CORES = 8
BPC = B // N_CORES           # samples per core
N_DVE = 14                   # taps computed on ACT+DVE instead of PE
N_DVE2 = 0                   # taps computed entirely on DVE (mul 2x + add)
N_PE = K * K - N_DVE - N_DVE2
SLEN = X * X                 # 961
TOFF = SLEN                  # raw template column section (49, padded to 64)
DOFF = SLEN + 64             # diag tiles for the PE taps
SECT = DOFF + N_PE * 128     # per-half section
R0, R1 = 13, 12              # output row split (13*26=338, 12*26=312 cols)
W = 26                       # even output window; col 25 is padding

_CACHE = {}


def _corr_half(nc, sb, ps, blob_s, h, out_view):
    """blob_s: [C, 2*SECT] tile; out_view: [C, OUT, OUT] slice of out_sb."""
    base = h * SECT
    sec = blob_s[:, base : base + SECT]
    d_v = sec[:, DOFF:].rearrange("c (k m) -> c k m", k=N_PE)
    f32 = mybir.dt.float32

    pa = ps.tile([C, R0, W], f32, tag="pa")
    pb = ps.tile([C, R1, W], f32, tag="pb")
    # PE: diag matmuls accumulating over taps 0..N_PE-1
    for (pt, r_base, nrows) in [(pa, 0, R0), (pb, R0, R1)]:
        for k in range(N_PE):
            di, dj = divmod(k, K)
            off = base + (r_base + di) * X + dj
            rows = blob_s[:, off : off + X * nrows].rearrange(
                "c (r j) -> c r j", j=X)[:, :, 0:W]
            nc.tensor.matmul(pt[:, :, :], d_v[:, k, :], rows,
                             start=(k == 0), stop=(k == N_PE - 1),
                             skip_group_check=True)
    # ACT+DVE: remaining taps as scale-multiplies + accumulate adds
    # (no even-count constraint here, so use the unpadded 25-wide window)
    acc = sb.tile([C, OUT, OUT], f32, tag="acc")
    for idx in range(N_DVE):
        k = N_PE + idx
        di, dj = divmod(k, K)
        off = base + di * X + dj
        rows = blob_s[:, off : off + X * OUT].rearrange(
            "c (r j) -> c r j", j=X)[:, :, 0:OUT].bitcast(f32)
        t_col = blob_s[:, base + TOFF + k : base + TOFF + k + 1].bitcast(f32)
        if idx == 0:
            nc.scalar.mul(acc[:, :, :], rows, t_col)
        else:
            tmp = sb.tile([C, OUT, OUT], f32, tag="tmp")
            nc.scalar.mul(tmp[:, :, :], rows, t_col)
            nc.vector.tensor_add(out=acc[:, :, :], in0=acc[:, :, :],
                                 in1=tmp[:, :, :])
    # DVE-only taps: tensor_scalar mul (2x mode, even 26-wide window) + add
    for idx2 in range(N_DVE2):
        k = N_PE + N_DVE + idx2
        di, dj = divmod(k, K)
        off = base + di * X + dj
        rows26 = blob_s[:, off : off + X * OUT].rearrange(
            "c (r j) -> c r j", j=X)[:, :, 0:W].bitcast(f32)
        t_col = blob_s[:, base + TOFF + k : base + TOFF + k + 1].bitcast(f32)
        tmp2 = sb.tile([C, OUT, W], f32, tag="tmp2")
        nc.vector.tensor_scalar_mul(tmp2[:, :, :], rows26, t_col)
        nc.vector.tensor_add(out=acc[:, :, :], in0=acc[:, :, :],
                             in1=tmp2[:, :, 0:OUT])
    # fold psum + acc -> out_sb
    nc.vector.tensor_add(out=out_view[:, 0:R0, :], in0=pa[:, :, 0:OUT],
                         in1=acc[:, 0:R0, :])
    nc.vector.tensor_add(out=out_view[:, R0:OUT, :], in0=pb[:, :, 0:OUT],
                         in1=acc[:, R0:OUT, :])


def _split_excess_waits(nc):
    """Walrus codegen allows a single sync-wait command per instruction.
    Move extra waits onto inserted same-engine NoOps; firing a monotone
    wait earlier on the same queue is always safe."""
    for fn in nc.m.functions:
        for bb in fn.blocks:
            out = []
            for inst in bb.instructions:
                si = inst.sync_info
                if si is not None and len(si.on_wait) > 1:
                    waits = list(si.on_wait)
                    for w in waits[:-1]:
                        nop = mybir.InstNoOp(
                            name=nc.get_next_instruction_name(), ins=[], outs=[])
                        nop.engine = inst.engine
                        nop.sync_info = mybir.SyncInfo(on_wait=[w], on_update=[])
                        out.append(nop)
                    si.on_wait = [waits[-1]]
                out.append(inst)
            bb.instructions = out


def _build_nc(reps=1):
    nc = bass.Bass("TRN2", debug=False)
    b_in = nc.dram_tensor("blob", [BPC, C, 2 * SECT], mybir.dt.float32r,
                          kind="ExternalInput").ap()
    o_out = nc.dram_tensor("o", [BPC, C, 2, OUT, OUT], mybir.dt.float32,
                           kind="ExternalOutput").ap()
    with tile.TileContext(nc) as tc:
        with tc.tile_pool(name="sb", bufs=3) as sb, \
             tc.tile_pool(name="work", bufs=3) as work, \
             tc.tile_pool(name="ps", bufs=2, space="PSUM") as ps:
            for _ in range(reps):
                for s in range(BPC):
                    blob_s = sb.tile([C, 2 * SECT], mybir.dt.float32r, tag="blob")
                    nc.sync.dma_start(out=blob_s[:], in_=b_in[s])
                    out_sb = work.tile([C, 2, OUT, OUT], mybir.dt.float32,
                                       tag="out_sb")
                    for h in range(2):
                        _corr_half(nc, work, ps, blob_s, h, out_sb[:, h])
                    nc.sync.dma_start(out=o_out[s], in_=out_sb[:])
    _split_excess_waits(nc)
    return nc


def _marshal(search, template):
    """-> blob [B, C, 2*SECT] float32."""
    search = np.ascontiguousarray(search, dtype=np.float32)
    template = np.ascontiguousarray(template, dtype=np.float32)
    s_cm = search.reshape(B, SLEN, 2, C).transpose(0, 2, 3, 1)     # [B,2,C,961]
    t_cm = template.reshape(B, K * K, 2, C).transpose(0, 2, 3, 1)  # [B,2,C,49]
    blob = np.zeros((B, 2, C, SECT), np.float32)
    blob[:, :, :, :SLEN] = s_cm
    blob[:, :, :, TOFF:TOFF + K * K] = t_cm
    d = blob[:, :, :, DOFF:].reshape(B, 2, C, N_PE, 128)
    c = np.arange(C)
    d[:, :, c, :, c] = t_cm[:, :, :, :N_PE].transpose(2, 0, 1, 3)
    # [B,2,C,SECT] -> [B,C,2*SECT]
    return np.ascontiguousarray(blob.transpose(0, 2, 1, 3).reshape(B, C, 2 * SECT))


def _unmarshal(results):
    o = np.stack([results[core]["o"] for core in range(N_CORES)])
    # [cores, BPC, C, 2, OUT, OUT] -> [B, OUT, OUT, 2, C] -> [B, OUT, OUT, CH]
    o = o.reshape(B, C, 2, OUT, OUT).transpose(0, 3, 4, 2, 1).reshape(B, OUT, OUT, CH)
    return np.ascontiguousarray(o)


def kernel(search, template):
    if "nc" not in _CACHE:
        _CACHE["nc"] = _build_nc()
    nc = _CACHE["nc"]
    blob = _marshal(search, template).reshape(N_CORES, BPC, C, 2 * SECT)
    in_maps = [{"blob": blob[core]} for core in range(N_CORES)]
    res = run_bass_kernel_spmd(nc, in_maps, core_ids=list(range(N_CORES)))
    return _unmarshal(res.results)



# revision 1
# speedup vs baseline: 1.0367x; 1.0367x over previous
"""Depthwise cross-correlation (per-sample dynamic kernel) on 8 Trainium2 cores.

reference: out[b,i,j,c] = sum_{di,dj} search[b,i+di,j+dj,c] * template[b,di,dj,c]
  search [64,31,31,256] f32, template [64,7,7,256] f32 -> out [64,25,25,256] f32

Strategy (pure data parallel, 8 samples/core, no collectives):
- Host marshals channel-major blobs, one per (sample): for each channel-half
  [128 part, 2 x (search 961 | raw template 49 | diag tiles for PE taps)].
- PE path: out_chunk = sum_k diag(t_k) @ shift_k(S) accumulated in PSUM
  (the only way a systolic array does depthwise). float32r dtype: fp32 bits
  at bf16-like speed, ~1e-4 rel err. fp32r needs even innermost counts and
  dst partition 0 -> full-height 128 diags, output window padded 25->26.
- ACT+DVE path: the last N_DVE taps run as ScalarE per-channel multiplies
  (activation Copy with per-partition scale AP) + VectorE accumulate adds,
  in parallel with the PE stream; folded with the PSUM result at evacuation.
- Output rows split 13+12 so each PSUM accumulation target is one bank with
  N>=256 (below 256 float32r drops to quarter rate).
- A post-pass splits multi-wait instructions (walrus allows one sync-wait
  per instruction) into single-wait NoOp carriers.
"""
import sys

sys.path.insert(0, "/opt/trn_rl_repo")

import numpy as np
import concourse.bass as bass
import concourse.mybir as mybir
import concourse.tile as tile
from concourse.bass_utils import run_bass_kernel_spmd

B = 64
X, K, OUT = 31, 7, 25
CH = 256
C = 128                      # channels per half (partition dim)
N_CORES = 8
BPC = B // N_CORES           # samples per core
N_DVE = 14                   # taps computed on ACT+DVE instead of PE
N_DVE2 = 0                   # taps computed entirely on DVE (mul 2x + add)
N_PE = K * K - N_DVE - N_DVE2
SLEN = X * X                 # 961
TOFF = SLEN                  # raw template column section (49, padded to 64)
DOFF = SLEN + 64             # diag tiles for the PE taps
SECT = DOFF + N_PE * 128     # per-half section
R0, R1 = 13, 12              # output row split (13*26=338, 12*26=312 cols)
W = 26                       # even output window; col 25 is padding

_CACHE = {}


def _corr_half(nc, sb, ps, blob_s, h, out_view):
    """blob_s: [C, 2*SECT] tile; out_view: [C, OUT, OUT] slice of out_sb."""
    base = h * SECT
    sec = blob_s[:, base : base + SECT]
    d_v = sec[:, DOFF:].rearrange("c (k m) -> c k m", k=N_PE)
    f32 = mybir.dt.float32

    pa = ps.tile([C, R0, W], f32, tag="pa")
    pb = ps.tile([C, R1, W], f32, tag="pb")
    # PE: diag matmuls accumulating over taps 0..N_PE-1
    for (pt, r_base, nrows) in [(pa, 0, R0), (pb, R0, R1)]:
        for k in range(N_PE):
            di, dj = divmod(k, K)
            off = base + (r_base + di) * X + dj
            rows = blob_s[:, off : off + X * nrows].rearrange(
                "c (r j) -> c r j", j=X)[:, :, 0:W]
            nc.tensor.matmul(pt[:, :, :], d_v[:, k, :], rows,
                             start=(k == 0), stop=(k == N_PE - 1),
                             skip_group_check=True)
    # ACT+DVE: remaining taps as scale-multiplies + accumulate adds
    # (no even-count constraint here, so use the unpadded 25-wide window)
    acc = sb.tile([C, OUT, OUT], f32, tag="acc")
    for idx in range(N_DVE):
        k = N_PE + idx
        di, dj = divmod(k, K)
        off = base + di * X + dj
        rows = blob_s[:, off : off + X * OUT].rearrange(
            "c (r j) -> c r j", j=X)[:, :, 0:OUT].bitcast(f32)
        t_col = blob_s[:, base + TOFF + k : base + TOFF + k + 1].bitcast(f32)
        if idx == 0:
            nc.scalar.mul(acc[:, :, :], rows, t_col)
        else:
            tmp = sb.tile([C, OUT, OUT], f32, tag="tmp")
            nc.scalar.mul(tmp[:, :, :], rows, t_col)
            nc.vector.tensor_add(out=acc[:, :, :], in0=acc[:, :, :],
                                 in1=tmp[:, :, :])
    # DVE-only taps: tensor_scalar mul (2x mode, even 26-wide window) + add
    for idx2 in range(N_DVE2):
        k = N_PE + N_DVE + idx2
        di, dj = divmod(k, K)
        off = base + di * X + dj
        rows26 = blob_s[:, off : off + X * OUT].rearrange(
            "c (r j) -> c r j", j=X)[:, :, 0:W].bitcast(f32)
        t_col = blob_s[:, base + TOFF + k : base + TOFF + k + 1].bitcast(f32)
        tmp2 = sb.tile([C, OUT, W], f32, tag="tmp2")
        nc.vector.tensor_scalar_mul(tmp2[:, :, :], rows26, t_col)
        nc.vector.tensor_add(out=acc[:, :, :], in0=acc[:, :, :],
                             in1=tmp2[:, :, 0:OUT])
    # fold psum + acc -> out_sb
    nc.vector.tensor_add(out=out_view[:, 0:R0, :], in0=pa[:, :, 0:OUT],
                         in1=acc[:, 0:R0, :])
    nc.vector.tensor_add(out=out_view[:, R0:OUT, :], in0=pb[:, :, 0:OUT],
                         in1=acc[:, R0:OUT, :])


def _split_excess_waits(nc):
    """Walrus codegen allows a single sync-wait command per instruction.
    Move extra waits onto inserted same-engine NoOps; firing a monotone
    wait earlier on the same queue is always safe."""
    for fn in nc.m.functions:
        for bb in fn.blocks:
            out = []
            for inst in bb.instructions:
                si = inst.sync_info
                if si is not None and len(si.on_wait) > 1:
                    waits = list(si.on_wait)
                    for w in waits[:-1]:
                        nop = mybir.InstNoOp(
                            name=nc.get_next_instruction_name(), ins=[], outs=[])
                        nop.engine = inst.engine
                        nop.sync_info = mybir.SyncInfo(on_wait=[w], on_update=[])
                        out.append(nop)
                    si.on_wait = [waits[-1]]
                out.append(inst)
            bb.instructions = out


def _build_nc(reps=1):
    nc = bass.Bass("TRN2", debug=False)
    b_in = nc.dram_tensor("blob", [BPC, C, 2 * SECT], mybir.dt.float32r,
                          kind="ExternalInput").ap()
    o_out = nc.dram_tensor("o", [BPC, C, 2, OUT, OUT], mybir.dt.float32,
                           kind="ExternalOutput").ap()
    with tile.TileContext(nc) as tc:
        with tc.tile_pool(name="sb", bufs=3) as sb, \
             tc.tile_pool(name="work", bufs=3) as work, \
             tc.tile_pool(name="ps", bufs=2, space="PSUM") as ps:
            for _ in range(reps):
                for s in range(BPC):
                    blob_s = sb.tile([C, 2 * SECT], mybir.dt.float32r, tag="blob")
                    nc.sync.dma_start(out=blob_s[:], in_=b_in[s])
                    out_sb = work.tile([C, 2, OUT, OUT], mybir.dt.float32,
                                       tag="out_sb")
                    for h in range(2):
                        _corr_half(nc, work, ps, blob_s, h, out_sb[:, h])
                    nc.sync.dma_start(out=o_out[s], in_=out_sb[:])
    _split_excess_waits(nc)
    return nc


def _marshal(search, template):
    """-> blob [B, C, 2*SECT] float32."""
    search = np.ascontiguousarray(search, dtype=np.float32)
    template = np.ascontiguousarray(template, dtype=np.float32)
    s_cm = search.reshape(B, SLEN, 2, C).transpose(0, 2, 3, 1)     # [B,2,C,961]
    t_cm = template.reshape(B, K * K, 2, C).transpose(0, 2, 3, 1)  # [B,2,C,49]
    blob = np.zeros((B, 2, C, SECT), np.float32)
    blob[:, :, :, :SLEN] = s_cm
    blob[:, :, :, TOFF:TOFF + K * K] = t_cm
    d = blob[:, :, :, DOFF:].reshape(B, 2, C, N_PE, 128)
    c = np.arange(C)
    d[:, :, c, :, c] = t_cm[:, :, :, :N_PE].transpose(2, 0, 1, 3)
    # [B,2,C,SECT] -> [B,C,2*SECT]
    return np.ascontiguousarray(blob.transpose(0, 2, 1, 3).reshape(B, C, 2 * SECT))


def _unmarshal(results):
    o = np.stack([results[core]["o"] for core in range(N_CORES)])
    # [cores, BPC, C, 2, OUT, OUT] -> [B, OUT, OUT, 2, C] -> [B, OUT, OUT, CH]
    o = o.reshape(B, C, 2, OUT, OUT).transpose(0, 3, 4, 2, 1).reshape(B, OUT, OUT, CH)
    return np.ascontiguousarray(o)


def kernel(search, template):
    if "nc" not in _CACHE:
        _CACHE["nc"] = _build_nc()
    nc = _CACHE["nc"]
    blob = _marshal(search, template).reshape(N_CORES, BPC, C, 2 * SECT)
    in_maps = [{"blob": blob[core]} for core in range(N_CORES)]
    res = run_bass_kernel_spmd(nc, in_maps, core_ids=list(range(N_CORES)))
    return _unmarshal(res.results)

